# revision 1
# baseline (speedup 1.0000x reference)
"""AtomMapContrastiveLoss Trainium2 Bass kernel.

Data-parallel over the graph dimension: each of the 8 NeuronCores handles
256 reactions (= 16384 atom rows), computes sum_{b,a}(logsumexp_c sim[b,a,c]
- sim[b,a,a]) for its slice, and the host averages the 8 partial sums.

Per-core pipeline (all shapes per chunk of 16 atom-groups = 2048 atoms):
  1. SWDGE cast-DMA: HBM f32 [2048, 256] -> SBUF fp16 natural [128, 16*256]
     (atom-within-group on partitions, (group, dim) on free axis).
  2. ACT Square -> DVE tensor_scalar accum (4x rate) -> per-atom sumsq.
  3. ACT: rinv = exp(-0.5*ln(sumsq) + bias)   (bias folds the 1/temperature
     into the reactant side; exp+ln share one ACT table set).
  4. DVE tensor_scalar: normalize in natural layout (per-partition scalar).
  5. HWDGE xbar transpose-DMA: [128, 16*256] -> [128, 32, 128] d-major blocks.
  6. PE fp16 matmuls, col-tiled two graphs per PSUM tile: sim/tau for 8
     graph-pairs batched in one [128, 512] PSUM bank.
  7. ACT exp (logits are bounded by 1/tau=10, so no max-subtraction),
     DVE segmented reduce -> softmax denominators, fused masked
     multiply-accumulate against a diagonal mask -> sum of diagonal logits.
  8. ln(S) + final reduction via ones-matmul -> scalar partial per core.

Schedule tuning (TimelineSim cost model, single serialized DMA resource):
the Square runs as two per-side ops so the reactant chain starts without
waiting for the product load; the chunk plan ends in half/quarter chunks
(16x7, 8, 4, 4) so the final chunk's load->square->reduce->rinv->normalize->
transpose chain drains quickly; xt/esc buffering is tuned so cast-loads and
xbar transposes stay interleaved on the DMA engines (157.5us -> 138.3us).
"""

import math
import os
from contextlib import ExitStack

import numpy as np

ATOMS = 64
GRAPHS = 2048
DIM = 256
N_CORES = 8
TAU = 0.1

GRAPHS_PER_CORE = GRAPHS // N_CORES          # 256
ROWS_PER_CORE = GRAPHS_PER_CORE * ATOMS      # 16384


def chunk_plan(rows, chunk_groups, tail_chunks, head_chunks=0):
    """Chunks of `chunk_groups` 128-atom groups; the first `head_chunks` and
    last `tail_chunks` worth are split into half-size chunks (head: first
    transpose is ready before the prefetch loads hog the DMA; tail: the final
    chunk's post-load chain drains faster). Returns [(gbase, ngroups), ...]."""
    n_groups = rows // 128
    assert n_groups % chunk_groups == 0
    plan = []
    g0 = 0
    for _ in range(2 * head_chunks):
        plan.append((g0, chunk_groups // 2))
        g0 += chunk_groups // 2
    full = n_groups // chunk_groups - tail_chunks - head_chunks
    for _ in range(full):
        plan.append((g0, chunk_groups))
        g0 += chunk_groups
    while g0 < n_groups:
        gsz = min(chunk_groups // 2, n_groups - g0)
        plan.append((g0, gsz))
        g0 += gsz
    return plan


def split_waits(nc, max_waits=1):
    """Split per-instruction semaphore waits beyond `max_waits` into
    standalone EventSemaphore instructions.

    The walrus build in this container accepts at most one sync-wait command
    per instruction; the Tile scheduler freely emits several. A sequencer
    stalls on a standalone EventSemaphore wait before dispatching subsequent
    instructions, so hoisting excess waits out is semantics-preserving.
    """
    from concourse import mybir

    n_split = 0
    for fn in nc.m.functions:
        for blk in fn.blocks:
            new_insts = []
            for inst in blk.instructions:
                si = inst.sync_info
                waits = list(si.on_wait) if si is not None and si.on_wait else []
                if len(waits) > max_waits and inst.opcode != "EventSemaphore":
                    keep = waits[:max_waits]
                    excess = waits[max_waits:]
                    for w in excess:
                        ev = mybir.InstEventSemaphore(
                            name=f"{inst.name}_wsplit{n_split}",
                            ins=[], outs=[], bass_nofuse=True,
                        )
                        ev.engine = inst.engine
                        ev.sync_info = mybir.SyncInfo(on_wait=[w], on_update=[])
                        new_insts.append(ev)
                        n_split += 1
                    inst.sync_info = mybir.SyncInfo(
                        on_wait=keep, on_update=list(si.on_update or [])
                    )
                new_insts.append(inst)
            blk.instructions = new_insts
    return n_split


def build_kernel(tc, out_ap, r_ap, p_ap, rows, chunk_groups=16,
                 nat_bufs=4, xt_bufs=2, scr_bufs=2, esc_bufs=4, psum_bufs=4,
                 split_q=False, split_t=False, rp_ap=None, tail_chunks=0,
                 split_l=False, split_q4=False, head_chunks=0, sqx_bufs=None,
                 oop_n=False, nmz_bufs=2, t_queue="sync", hp_loads=False,
                 mes_last=False, plan_override=None, ppb=8, sq_bufs=None,
                 chunk_lnd=False, split_nat=False, dmask_late=False,
                 fold_sub=False):
    """Trace the per-core kernel into TileContext `tc`.

    rows: atom rows this core handles (rows % 128 == 0).
    chunk_groups: 128-atom groups per DMA chunk.
    """
    import concourse.bass as bass
    from concourse import mybir

    nc = tc.nc
    f32 = mybir.dt.float32
    f16 = mybir.dt.float16
    i32 = mybir.dt.int32
    Alu = mybir.AluOpType
    Act = mybir.ActivationFunctionType

    if split_nat:
        split_q = True
        split_t = True
    n_groups = rows // 128                     # one group = 128 atoms = 2 graphs
    if plan_override is not None:
        plan = []
        g0 = 0
        for gsz in plan_override:
            plan.append((g0, gsz))
            g0 += gsz
        assert g0 == rows // 128, plan
    else:
        plan = chunk_plan(rows, chunk_groups, tail_chunks, head_chunks)
    PPB = ppb                                  # graph-pairs per PSUM batch
    for _, gsz in plan:
        assert gsz % min(PPB, gsz) == 0, plan
    n_batches = sum(max(1, gsz // PPB) for _, gsz in plan)
    scol = []
    _g = 0
    for _, gsz in plan:
        _p = min(PPB, gsz)
        for _b in range(gsz // _p):
            scol.append(_g)
            _g += _p
    assert _g == rows // 128
    ln10 = math.log(1.0 / TAU)

    with ExitStack() as ctx:
        singles = ctx.enter_context(tc.tile_pool(name="singles", bufs=1))
        nat_pool = ctx.enter_context(tc.tile_pool(name="nat", bufs=nat_bufs))
        natp_pool = (ctx.enter_context(tc.tile_pool(name="natp",
                                                    bufs=nat_bufs))
                     if split_nat else None)
        xtp_pool = (ctx.enter_context(tc.tile_pool(name="xtp", bufs=xt_bufs))
                    if split_nat else None)
        nmz_pool = (ctx.enter_context(tc.tile_pool(name="nmz", bufs=nmz_bufs))
                    if oop_n else None)
        xt_pool = ctx.enter_context(tc.tile_pool(name="xt", bufs=xt_bufs))
        scr_pool = ctx.enter_context(tc.tile_pool(name="scr", bufs=scr_bufs))
        esc_pool = ctx.enter_context(tc.tile_pool(name="esc", bufs=esc_bufs))
        psum_pool = ctx.enter_context(
            tc.tile_pool(name="psum", bufs=psum_bufs, space="PSUM")
        )

        # ---- one-time constants -------------------------------------------
        n2_r = singles.tile([128, n_groups], f32, name="n2_r")
        n2_p = singles.tile([128, n_groups], f32, name="n2_p")
        rinv_r = singles.tile([128, n_groups], f32, name="rinv_r")
        rinv_p = singles.tile([128, n_groups], f32, name="rinv_p")
        s_all = singles.tile([128, n_groups], f32, name="s_all")
        td_all = singles.tile([128, n_batches], f32, name="td_all")
        sums = singles.tile([128, 4], f32, name="sums")
        sums_c = singles.tile([128, len(plan)], f32, name="sums_c")
        ones = singles.tile([128, 1], f32, name="ones")
        lnd = singles.tile([128, n_groups], f32, name="lnd")
        res_sb = singles.tile([1, 1], f32, name="res_sb")

        nc.vector.memset(ones[:], 1.0)
        bias10 = singles.tile([128, 1], f32, name="bias10")
        nc.vector.memset(bias10[:], ln10)

        # Diagonal mask [128, PPB*64]: 1.0 where col-within-64-block == row%64.
        p_idx = np.arange(128) % 64
        c_idx = np.tile(np.arange(64), PPB)
        mask_np = (c_idx[None, :] == p_idx[:, None]).astype(np.float32)
        dmask_dram = nc.inline_tensor(mask_np, name="dmask_const")
        dmask = singles.tile([128, PPB * 64], f32, name="dmask")
        if not dmask_late:
            nc.sync.dma_start(out=dmask[:], in_=dmask_dram.ap())

        # ---- main loop over chunks ----------------------------------------
        b_glob = 0
        deferred_mes = []
        for c, (gbase, G) in enumerate(plan):
            CA = G * 128
            csl = slice(gbase, gbase + G)
            ppb_c = min(PPB, G)
            batches_per_chunk = G // ppb_c

            # 1. cast-DMA loads (f32 HBM -> fp16 SBUF natural layout).
            # split_nat: r and p in separate tiles so the r stream's WAR
            # release (and its transpose) never waits on the p stream.
            if split_nat:
                nat_rp = None
                nat_r_t = nat_pool.tile([128, G * 256], f16, name="nat_r",
                                        tag="nat_r")
                nat_p_t = natp_pool.tile([128, G * 256], f16, name="nat_p",
                                         tag="nat_p")
                nat_r = nat_r_t[:]
                nat_p = nat_p_t[:]
            else:
                nat_rp = nat_pool.tile([128, 2 * G * 256], f16, name="nat_rp",
                                       tag="nat_rp")
                nat_r = nat_rp[:, :G * 256]
                nat_p = nat_rp[:, G * 256:]
            import contextlib
            hp_cm = tc.high_priority() if hp_loads else contextlib.nullcontext()
            with hp_cm:
             if rp_ap is not None:
                # merged chunk-interleaved input tensor (host lays out each
                # chunk as [r-chunk; p-chunk] contiguously): one cast-DMA per
                # chunk covers both sides with a uniform 3-D access pattern.
                off = 2 * gbase * 128
                src = rp_ap[off:off + 2 * CA, :].rearrange(
                    "(g p) d -> p g d", p=128
                )
                nc.gpsimd.dma_start(
                    out=nat_rp[:].rearrange("p (g d) -> p g d", d=256),
                    in_=src,
                )
             elif split_l and G > 1:
                # half-size load DMAs: finer slices interleave with transposes
                Gh = G // 2
                for side_ap, dst in ((r_ap, nat_r), (p_ap, nat_p)):
                    for hh in range(2):
                        rsl = slice(gbase * 128 + hh * Gh * 128,
                                    gbase * 128 + (hh + 1) * Gh * 128)
                        nc.gpsimd.dma_start(
                            out=dst[:, hh * Gh * 256:(hh + 1) * Gh * 256]
                                .rearrange("p (g d) -> p g d", d=256),
                            in_=side_ap[rsl, :].rearrange(
                                "(g p) d -> p g d", p=128),
                        )
             else:
                src_r = r_ap[gbase * 128:gbase * 128 + CA, :].rearrange(
                    "(g p) d -> p g d", p=128
                )
                src_p = p_ap[gbase * 128:gbase * 128 + CA, :].rearrange(
                    "(g p) d -> p g d", p=128
                )
                nc.gpsimd.dma_start(
                    out=nat_r[:].rearrange("p (g d) -> p g d", d=256), in_=src_r
                )
                nc.gpsimd.dma_start(
                    out=nat_p[:].rearrange("p (g d) -> p g d", d=256), in_=src_p
                )

            if dmask_late and c == 0:
                nc.sync.dma_start(out=dmask[:], in_=dmask_dram.ap())

            # 2. per-atom sum of squares: batched ACT square (same table set
            # as Exp/Ln), then per-group DVE accumulating reduce at 4x rate.
            sqx_rp = scr_pool.tile([128, 2 * G * 256], f16, name="sqx_rp",
                                    tag="sqx_rp", bufs=sqx_bufs)
            sqx_r = sqx_rp[:, :G * 256]
            sqx_p = sqx_rp[:, G * 256:]
            if split_q4 and G > 1:
                Gh = G // 2
                for qq in range(4):
                    qsl = slice(qq * Gh * 256, (qq + 1) * Gh * 256)
                    nc.scalar.activation(out=sqx_rp[:, qsl],
                                         in_=nat_rp[:, qsl], func=Act.Square)
            elif split_q:
                nc.scalar.activation(out=sqx_r[:], in_=nat_r[:],
                                     func=Act.Square)
                nc.scalar.activation(out=sqx_p[:], in_=nat_p[:],
                                     func=Act.Square)
            else:
                nc.scalar.activation(out=sqx_rp[:], in_=nat_rp[:],
                                     func=Act.Square)
            for g in range(G):
                gs = slice(g * 256, (g + 1) * 256)
                col = gbase + g
                sq = scr_pool.tile([128, 256], f16, name="sq", tag="sq",
                                   bufs=sq_bufs)
                nc.vector.tensor_scalar(
                    out=sq[:], in0=sqx_r[:, gs], scalar1=1.0, scalar2=0.0,
                    op0=Alu.mult, op1=Alu.add, accum_out=n2_r[:, col:col + 1],
                )
                sq2 = scr_pool.tile([128, 256], f16, name="sq2", tag="sq",
                                    bufs=sq_bufs)
                nc.vector.tensor_scalar(
                    out=sq2[:], in0=sqx_p[:, gs], scalar1=1.0, scalar2=0.0,
                    op0=Alu.mult, op1=Alu.add, accum_out=n2_p[:, col:col + 1],
                )

            # 3. rinv = exp(-0.5 * ln(n2) + bias); reactant side folds 1/tau
            lnt_r = scr_pool.tile([128, G], f32, name="lnt_r", tag="lnt")
            lnt_p = scr_pool.tile([128, G], f32, name="lnt_p", tag="lnt")
            nc.scalar.activation(out=lnt_r[:], in_=n2_r[:, csl], func=Act.Ln)
            nc.scalar.activation(
                out=rinv_r[:, csl], in_=lnt_r[:], func=Act.Exp,
                scale=-0.5, bias=bias10[:, 0:1],
            )
            nc.scalar.activation(out=lnt_p[:], in_=n2_p[:, csl], func=Act.Ln)
            nc.scalar.activation(
                out=rinv_p[:, csl], in_=lnt_p[:], func=Act.Exp,
                scale=-0.5, bias=0.0,
            )

            # 4. normalize in natural layout (per-partition scalars).
            # With oop_n the result goes to a separate tile so the nat buffer
            # frees at normalize time: the next chunk-load's SWDGE descriptor
            # generation then overlaps the transpose instead of trailing it.
            if oop_n:
                nmz_rp = nmz_pool.tile([128, 2 * G * 256], f16, name="nmz_rp",
                                       tag="nmz_rp")
                dst_r = nmz_rp[:, :G * 256]
                dst_p = nmz_rp[:, G * 256:]
            else:
                nmz_rp, dst_r, dst_p = nat_rp, nat_r, nat_p
            for g in range(G):
                gs = slice(g * 256, (g + 1) * 256)
                col = gbase + g
                nc.vector.tensor_scalar_mul(
                    out=dst_r[:, gs], in0=nat_r[:, gs],
                    scalar1=rinv_r[:, col:col + 1],
                )
                nc.vector.tensor_scalar_mul(
                    out=dst_p[:, gs], in0=nat_p[:, gs],
                    scalar1=rinv_p[:, col:col + 1],
                )

            # 5. xbar transpose (one op, or per side when split_t)
            if split_nat:
                xt_r_t = xt_pool.tile([128, 2 * G, 128], f16, name="xt_r",
                                      tag="xt_r")
                xt_p_t = xtp_pool.tile([128, 2 * G, 128], f16, name="xt_p",
                                       tag="xt_p")
                xt_r = xt_r_t[:]
                xt_p = xt_p_t[:]
            else:
                xt_rp = xt_pool.tile([128, 4 * G, 128], f16, name="xt_rp",
                                     tag="xt_rp")
                xt_r = xt_rp[:, :2 * G, :]
                xt_p = xt_rp[:, 2 * G:, :]
            t_eng = getattr(nc, {"sync": "sync", "vector": "vector",
                                 "act": "scalar"}[t_queue])
            if split_t:
                t_eng.dma_start_transpose(out=xt_r[:], in_=dst_r[:])
                t_eng.dma_start_transpose(out=xt_p[:], in_=dst_p[:])
            else:
                t_eng.dma_start_transpose(out=xt_rp[:], in_=nmz_rp[:])

            # 6+7. sim matmuls, exp, denominators, diagonal extraction
            def stage_mes(xt_r, xt_p, b0, batches_per_chunk, ppb_c):
              for b in range(batches_per_chunk):
                  b_glob = b0 + b
                  s0 = scol[b_glob]
                  pt = psum_pool.tile([128, ppb_c * 64], f32, name="pt",
                                      tag="pt")
                  for q in range(ppb_c):
                      g = b * ppb_c + q
                      cols = slice(q * 64, q * 64 + 64)
                      blk0, blk1 = 2 * g, 2 * g + 1
                      # even graph of the pair -> output partitions 0..63
                      nc.tensor.matmul(
                          pt[0:64, cols], xt_r[:, blk0, 0:64], xt_p[:, blk0, 0:64],
                          start=True, stop=False, tile_position=(0, 0),
                      )
                      nc.tensor.matmul(
                          pt[0:64, cols], xt_r[:, blk1, 0:64], xt_p[:, blk1, 0:64],
                          start=False, stop=True, tile_position=(0, 0),
                      )
                      # odd graph -> output partitions 64..127
                      nc.tensor.matmul(
                          pt[64:128, cols], xt_r[:, blk0, 64:128],
                          xt_p[:, blk0, 64:128],
                          start=True, stop=False, tile_position=(0, 64),
                      )
                      nc.tensor.matmul(
                          pt[64:128, cols], xt_r[:, blk1, 64:128],
                          xt_p[:, blk1, 64:128],
                          start=False, stop=True, tile_position=(0, 64),
                      )

                  esc = esc_pool.tile([128, ppb_c * 64], f32, name="esc", tag="esc")
                  nc.scalar.activation(out=esc[:], in_=pt[:], func=Act.Exp)
                  nc.vector.reduce_sum(
                      out=s_all[:, s0:s0 + ppb_c],
                      in_=esc[:].rearrange("p (j c) -> p j c", c=64),
                      axis=mybir.AxisListType.X,
                  )
                  dum = esc_pool.tile([128, ppb_c * 64], f16, name="dum", tag="dum")
                  nc.vector.scalar_tensor_tensor(
                      out=dum[:], in0=pt[:], scalar=1.0, in1=dmask[:, :ppb_c * 64],
                      op0=Alu.mult, op1=Alu.mult,
                      accum_out=td_all[:, b_glob:b_glob + 1],
                  )

            if mes_last:
                deferred_mes.append((xt_r, xt_p, b_glob, batches_per_chunk,
                                     ppb_c))
            else:
                stage_mes(xt_r, xt_p, b_glob, batches_per_chunk, ppb_c)
                if chunk_lnd:
                    nc.scalar.activation(
                        out=lnd[:, csl], in_=s_all[:, csl], func=Act.Ln,
                        accum_out=sums_c[:, c:c + 1],
                    )
            b_glob += batches_per_chunk

        for args in deferred_mes:
            stage_mes(*args)

        # ---- 8. final reduction -------------------------------------------
        if chunk_lnd:
            nc.vector.reduce_sum(
                out=sums[:, 0:1], in_=sums_c[:], axis=mybir.AxisListType.X
            )
        else:
            nc.scalar.activation(
                out=lnd[:], in_=s_all[:], func=Act.Ln, accum_out=sums[:, 0:1]
            )
        nc.vector.reduce_sum(
            out=sums[:, 1:2], in_=td_all[:], axis=mybir.AxisListType.X
        )
        if fold_sub:
            # sums2 = (-1 * sums1) + sums0 in one DVE op
            nc.vector.scalar_tensor_tensor(
                out=sums[:, 2:3], in0=sums[:, 1:2], scalar=-1.0,
                in1=sums[:, 0:1], op0=Alu.mult, op1=Alu.add,
            )
        else:
            nc.vector.tensor_tensor(
                out=sums[:, 2:3], in0=sums[:, 0:1], in1=sums[:, 1:2],
                op=Alu.subtract,
            )
        res_ps = psum_pool.tile([1, 1], f32, name="res_ps", tag="res", bufs=1)
        nc.tensor.matmul(res_ps[:], ones[:, 0:1], sums[:, 2:3])
        nc.vector.tensor_copy(out=res_sb[:], in_=res_ps[:])
        nc.sync.dma_start(out=out_ap, in_=res_sb[:])


def build_kernel_fp8(tc, out_ap, r_ap, p_ap, rows, chunk_groups=16,
                     nat_bufs=4, xt_bufs=4, scr_bufs=2, esc_bufs=6,
                     psum_bufs=3, plan_override=None, rp_ap=None,
                     no_scale_mm=False):
    """fp8 variant: loads cast f32->fp8e4 (priced on the f32 source's 1 KB
    runs -> ~half the fp16 load cost), one u16 transpose moves DIM-PAIRS
    (half the xbar tiles), matmuls contract even/odd dims via byte-strided
    fp8 access patterns. No pre-matmul normalization: PE builds two scale
    tiles per batch (rows: rinv_r via block-indicator matmul; columns:
    10*rinv_p via a two-row indicator matmul), DVE multiplies them into the
    raw PSUM logits, and one batch-level Exp + masked reduce finish.
    """
    import concourse.bass as bass
    from concourse import mybir

    nc = tc.nc
    f32 = mybir.dt.float32
    f16 = mybir.dt.float16
    u16 = mybir.dt.uint16
    f8 = mybir.dt.float8e4
    Alu = mybir.AluOpType
    Act = mybir.ActivationFunctionType

    n_groups = rows // 128
    if plan_override is None:
        plan_override = (chunk_groups,) * (n_groups // chunk_groups)
    plan = []
    g0 = 0
    for gsz in plan_override:
        plan.append((g0, gsz))
        g0 += gsz
    assert g0 == n_groups, plan
    PPB = 8
    for _, gsz in plan:
        assert gsz % min(PPB, gsz) == 0, plan
    n_batches = sum(max(1, gsz // PPB) for _, gsz in plan)
    scol = []
    _g = 0
    for _, gsz in plan:
        _p = min(PPB, gsz)
        for _b in range(gsz // _p):
            scol.append(_g)
            _g += _p

    from contextlib import ExitStack
    with ExitStack() as ctx:
        singles = ctx.enter_context(tc.tile_pool(name="singles", bufs=1))
        nat_pool = ctx.enter_context(tc.tile_pool(name="nat", bufs=nat_bufs))
        xt_pool = ctx.enter_context(tc.tile_pool(name="xt", bufs=xt_bufs))
        scr_pool = ctx.enter_context(tc.tile_pool(name="scr", bufs=scr_bufs))
        esc_pool = ctx.enter_context(tc.tile_pool(name="esc", bufs=esc_bufs))
        psum_pool = ctx.enter_context(
            tc.tile_pool(name="psum", bufs=psum_bufs, space="PSUM")
        )

        n2_r = singles.tile([128, n_groups], f32, name="n2_r")
        n2_p = singles.tile([128, n_groups], f32, name="n2_p")
        rinv_r = singles.tile([128, n_groups], f16, name="rinv_r")
        rinv_p = singles.tile([128, n_groups], f16, name="rinv_p")
        s_all = singles.tile([128, n_groups], f32, name="s_all")
        td_all = singles.tile([128, n_batches], f32, name="td_all")
        sums = singles.tile([128, 4], f32, name="sums")
        ones = singles.tile([128, 1], f32, name="ones")
        lnd = singles.tile([128, n_groups], f32, name="lnd")
        res_sb = singles.tile([1, 1], f32, name="res_sb")

        nc.vector.memset(ones[:], 1.0)

        # all constants in one inline tensor / one DMA (lead-in matters):
        # cols 0:512 = diagonal mask; 512:1024 (rows<64) = block indicator;
        # 1024:1152 (rows<2) = x10 two-row indicator (folds 1/tau)
        cst = np.zeros((128, 1152), dtype=np.float16)
        p_idx = np.arange(128) % 64
        c_idx = np.tile(np.arange(64), PPB)
        cst[:, :512] = (c_idx[None, :] == p_idx[:, None])
        for q in range(8):
            cst[q, 512 + q * 64:512 + (q + 1) * 64] = 1.0
            cst[64 + q, 512 + q * 64:512 + (q + 1) * 64] = 1.0
        cst[0, 1024:1088] = 10.0
        cst[1, 1088:1152] = 10.0
        cst_dram = nc.inline_tensor(cst, name="consts")
        consts = singles.tile([128, 1152], f16, name="consts")
        nc.sync.dma_start(out=consts[:], in_=cst_dram.ap())
        dmask = consts[:, 0:512]
        bi = consts[:, 512:1024]
        ind2 = consts[0:2, 1024:1152].rearrange("k (h a) -> k h a", h=1)

        b_glob = 0
        for c, (gbase, G) in enumerate(plan):
            CA = G * 128
            csl = slice(gbase, gbase + G)
            ppb_c = min(PPB, G)
            batches_per_chunk = G // ppb_c

            # 1. cast loads f32 -> fp8 (u16-typed tile holds dim pairs)
            nat_u = nat_pool.tile([128, 2 * G * 128], u16, name="nat_u",
                                  tag="nat_u")
            nat8 = nat_u[:].bitcast(f8)
            if rp_ap is not None:
                # merged chunk-interleaved layout: one gen+transfer per chunk
                off = 2 * gbase * 128
                nc.gpsimd.dma_start(
                    out=nat8[:].rearrange("p (g d) -> p g d", d=256),
                    in_=rp_ap[off:off + 2 * CA, :].rearrange(
                        "(g p) d -> p g d", p=128))
            else:
                src_r = r_ap[gbase * 128:gbase * 128 + CA, :].rearrange(
                    "(g p) d -> p g d", p=128)
                src_p = p_ap[gbase * 128:gbase * 128 + CA, :].rearrange(
                    "(g p) d -> p g d", p=128)
                nc.gpsimd.dma_start(
                    out=nat8[:, :G * 256].rearrange("p (g d) -> p g d", d=256),
                    in_=src_r)
                nc.gpsimd.dma_start(
                    out=nat8[:, G * 256:].rearrange("p (g d) -> p g d", d=256),
                    in_=src_p)

            # 2. sumsq from the quantized data (per-side ACT squares)
            sqx = scr_pool.tile([128, 2 * G * 256], f16, name="sqx",
                                tag="sqx")
            nc.scalar.activation(out=sqx[:, :G * 256], in_=nat8[:, :G * 256],
                                 func=Act.Square)
            nc.scalar.activation(out=sqx[:, G * 256:], in_=nat8[:, G * 256:],
                                 func=Act.Square)
            for g in range(G):
                gs = slice(g * 256, (g + 1) * 256)
                col = gbase + g
                sq = scr_pool.tile([128, 256], f16, name="sq", tag="sq")
                nc.vector.tensor_scalar(
                    out=sq[:], in0=sqx[:, gs], scalar1=1.0, scalar2=0.0,
                    op0=Alu.mult, op1=Alu.add, accum_out=n2_r[:, col:col + 1])
                gp = slice((G + g) * 256, (G + g + 1) * 256)
                sq2 = scr_pool.tile([128, 256], f16, name="sq2", tag="sq")
                nc.vector.tensor_scalar(
                    out=sq2[:], in0=sqx[:, gp], scalar1=1.0, scalar2=0.0,
                    op0=Alu.mult, op1=Alu.add, accum_out=n2_p[:, col:col + 1])

            # 3. rinv (both plain f16; the 1/tau lives in ind2)
            lnt_r = scr_pool.tile([128, G], f32, name="lnt_r", tag="lnt")
            lnt_p = scr_pool.tile([128, G], f32, name="lnt_p", tag="lnt")
            # rinv_r goes straight into the pre-transpose layout: columns
            # 32*b + q so transposed rows land at base partitions {0, 32}
            nc.scalar.activation(out=lnt_r[:], in_=n2_r[:, csl], func=Act.Ln)
            with nc.allow_low_precision(reason="fp16 inverse norms"):
                nc.scalar.activation(out=rinv_r[:, csl], in_=lnt_r[:],
                                     func=Act.Exp, scale=-0.5)
                nc.scalar.activation(out=lnt_p[:], in_=n2_p[:, csl],
                                     func=Act.Ln)
                nc.scalar.activation(out=rinv_p[:, csl], in_=lnt_p[:],
                                     func=Act.Exp, scale=-0.5)

            # 3b. scale tiles built directly with partition-traversing
            # element DMAs (one op each; priced at the 7 ns/desc floor).
            # rTr[32b+q, a] = rinv_r[atom a, group b*ppb+q]
            rTr = scr_pool.tile([128, 128], f16, name="rTr", tag="rTr")
            for b in range(batches_per_chunk):
                nc.sync.dma_start(
                    out=rTr[64 * b:64 * b + ppb_c, :],
                    in_=rinv_r[:, gbase + b * ppb_c:
                               gbase + (b + 1) * ppb_c].rearrange(
                                   "a q -> q a"))
            # tRows[h, g*64+c] = rinv_p[atom 64h+c, group g]
            tRows = scr_pool.tile([2, G * 64], f16, name="tRows", tag="tRows")
            for h in range(2):
                nc.sync.dma_start(
                    out=tRows[h:h + 1, :].rearrange("x (g c) -> (x g) c",
                                                    c=64),
                    in_=rinv_p[64 * h:64 * h + 64, csl].rearrange(
                        "c g -> g c"))

            # 4. one u16 transpose: [128, 2G*128] -> [128, 2G, 128]
            xt_u = xt_pool.tile([128, 2 * G, 128], u16, name="xt_u",
                                tag="xt_u")
            nc.sync.dma_start_transpose(out=xt_u[:], in_=nat_u[:])
            x8 = xt_u[:].bitcast(f8).rearrange(
                "p b (a two) -> p b a two", two=2)

            # 5. all raw sim matmuls first: they depend only on the
            # transpose, so PE never head-blocks on the scale-constant chain
            W = ppb_c * 64
            pts = []
            for b in range(batches_per_chunk):
                pt = psum_pool.tile([128, W], f32, name="pt", tag="pt")
                for q in range(ppb_c):
                    g = b * ppb_c + q
                    cols = slice(q * 64, q * 64 + 64)
                    bp = G + g
                    for half, rsl in ((0, slice(0, 64)), (64, slice(64, 128))):
                        for parity in (0, 1):
                            nc.tensor.matmul(
                                pt[half:half + 64, cols],
                                x8[:, g, rsl, parity],
                                x8[:, bp, rsl, parity],
                                start=(parity == 0), stop=(parity == 1),
                                tile_position=(0, half),
                            )
                pts.append(pt)
            # 6-7. per batch: scale tiles, combined scale, exp, reduces
            for b in range(batches_per_chunk):
                s0 = scol[b_glob]
                pt = pts[b]
                ssim = esc_pool.tile([128, W], f16, name="ssim", tag="ssim")
                if no_scale_mm:
                    # bisection control: drop the scale matmuls entirely
                    with nc.allow_low_precision(reason="bisection control"):
                        nc.vector.tensor_copy(out=ssim[:], in_=pt[:])
                else:
                    trep = psum_pool.tile([128, W], f32, name="trep",
                                          tag="trep", bufs=2)
                    nc.tensor.matmul(
                        trep[:], ind2[:, 0, :],
                        tRows[0:2, b * W:(b + 1) * W])
                    wr = psum_pool.tile([128, W], f32, name="wr", tag="wr",
                                        bufs=2)
                    nc.tensor.matmul(
                        wr[:], rTr[64 * b:64 * b + ppb_c, 0:128],
                        bi[64 * b:64 * b + ppb_c, 0:W])
                    with nc.allow_low_precision(reason="scale products fp16"):
                        w = esc_pool.tile([128, W], f16, name="w", tag="w")
                        nc.vector.tensor_tensor(out=w[:], in0=trep[:],
                                                in1=wr[:], op=Alu.mult)
                    nc.vector.tensor_tensor(out=ssim[:], in0=pt[:], in1=w[:],
                                            op=Alu.mult)
                esc = esc_pool.tile([128, W], f16, name="esc", tag="esc")
                nc.scalar.activation(out=esc[:], in_=ssim[:], func=Act.Exp)
                with nc.allow_low_precision(reason="denominators bounded "
                                            "by 64*e^10 fit fp16 via f32 out"):
                    nc.vector.reduce_sum(
                        out=s_all[:, s0:s0 + ppb_c],
                        in_=esc[:].rearrange("p (j c) -> p j c", c=64),
                        axis=mybir.AxisListType.X)
                dum = esc_pool.tile([128, W], f16, name="dum", tag="dum")
                nc.vector.scalar_tensor_tensor(
                    out=dum[:], in0=ssim[:], scalar=1.0, in1=dmask[:, :W],
                    op0=Alu.mult, op1=Alu.mult,
                    accum_out=td_all[:, b_glob:b_glob + 1])
                b_glob += 1

        # final: partial = sum(ln S) - sum(diag logits)
        nc.scalar.activation(out=lnd[:], in_=s_all[:], func=Act.Ln,
                             accum_out=sums[:, 0:1])
        nc.vector.reduce_sum(out=sums[:, 1:2], in_=td_all[:],
                             axis=mybir.AxisListType.X)
        nc.vector.tensor_tensor(out=sums[:, 2:3], in0=sums[:, 0:1],
                                in1=sums[:, 1:2], op=Alu.subtract)
        res_ps = psum_pool.tile([1, 1], f32, name="res_ps", tag="res", bufs=1)
        nc.tensor.matmul(res_ps[:], ones[:, 0:1], sums[:, 2:3])
        nc.vector.tensor_copy(out=res_sb[:], in_=res_ps[:])
        nc.sync.dma_start(out=out_ap, in_=res_sb[:])


def _build_nc(rows=ROWS_PER_CORE, chunk_groups=16, merged_load=False,
              max_waits=1, dma_scratch=16384, **kw):
    import concourse.bass as bass
    import concourse.tile as tile
    from concourse import mybir

    nc = bass.Bass(
        "TRN2", target_bir_lowering=False, debug=False, num_devices=N_CORES,
        dynamic_dma_scratch_size=dma_scratch,
    )
    out = nc.dram_tensor("partial_out", [1, 1], mybir.dt.float32,
                         kind="ExternalOutput")
    fp8 = kw.pop("fp8", False)
    with tile.TileContext(nc) as tc:
        if fp8:
            if merged_load:
                rp = nc.dram_tensor("rp_in", [2 * rows, DIM],
                                    mybir.dt.float32, kind="ExternalInput")
                build_kernel_fp8(tc, out.ap(), None, None, rows,
                                 chunk_groups, rp_ap=rp.ap(), **kw)
            else:
                r = nc.dram_tensor("r_in", [rows, DIM], mybir.dt.float32,
                                   kind="ExternalInput")
                p = nc.dram_tensor("p_in", [rows, DIM], mybir.dt.float32,
                                   kind="ExternalInput")
                build_kernel_fp8(tc, out.ap(), r.ap(), p.ap(), rows,
                                 chunk_groups, **kw)
        elif merged_load:
            rp = nc.dram_tensor("rp_in", [2 * rows, DIM], mybir.dt.float32,
                                kind="ExternalInput")
            build_kernel(tc, out.ap(), None, None, rows, chunk_groups,
                         rp_ap=rp.ap(), **kw)
        else:
            r = nc.dram_tensor("r_in", [rows, DIM], mybir.dt.float32,
                               kind="ExternalInput")
            p = nc.dram_tensor("p_in", [rows, DIM], mybir.dt.float32,
                               kind="ExternalInput")
            build_kernel(tc, out.ap(), r.ap(), p.ap(), rows, chunk_groups,
                         **kw)
    split_waits(nc, max_waits=max_waits)
    return nc


# Build configuration used by kernel(); sweep scripts override _build_nc args
# directly instead.
BEST_CONFIG = dict(split_q=True, xt_bufs=4, esc_bufs=6,
                   plan_override=(16,)*7 + (8, 4, 4))
_NC_CACHE = None


def make_in_maps(r, p, merged_load=False, chunk_groups=16, tail_chunks=0):
    in_maps = []
    plan = chunk_plan(ROWS_PER_CORE, chunk_groups, tail_chunks)
    for c in range(N_CORES):
        sl = slice(c * ROWS_PER_CORE, (c + 1) * ROWS_PER_CORE)
        if merged_load:
            rc, pc = r[sl], p[sl]
            blocks = []
            for gbase, gsz in plan:
                rsl = slice(gbase * 128, gbase * 128 + gsz * 128)
                blocks.append(rc[rsl])
                blocks.append(pc[rsl])
            in_maps.append({
                "rp_in": np.ascontiguousarray(np.concatenate(blocks, axis=0)),
            })
        else:
            in_maps.append({
                "r_in": np.ascontiguousarray(r[sl]),
                "p_in": np.ascontiguousarray(p[sl]),
            })
    return in_maps


def kernel(reactant_features, product_features,
           reactant_batch_indices=None, product_batch_indices=None):
    """Full-input entry point: shards over 8 NeuronCores internally."""
    global _NC_CACHE
    # Persistent JAX compilation cache so repeat invocations skip neuronxcc.
    os.environ.setdefault("JAX_COMPILATION_CACHE_DIR", "/root/.cache/jax_bass")
    import jax
    try:
        jax.config.update("jax_compilation_cache_dir",
                          os.environ["JAX_COMPILATION_CACHE_DIR"])
    except Exception:
        pass

    from concourse.bass_utils import run_bass_kernel_spmd

    r = np.asarray(reactant_features, dtype=np.float32)
    p = np.asarray(product_features, dtype=np.float32)
    assert r.shape == (GRAPHS * ATOMS, DIM), r.shape

    if _NC_CACHE is None:
        _NC_CACHE = _build_nc(**BEST_CONFIG)
    nc = _NC_CACHE

    in_maps = make_in_maps(r, p,
                           BEST_CONFIG.get("merged_load", False),
                           BEST_CONFIG.get("chunk_groups", 16),
                           BEST_CONFIG.get("tail_chunks", 0))

    res = run_bass_kernel_spmd(nc, in_maps, core_ids=list(range(N_CORES)))
    total = 0.0
    for c in range(N_CORES):
        total += float(res.results[c]["partial_out"][0, 0])
    loss = total / float(GRAPHS * ATOMS)
    return np.float32(loss)

